# revision 36
# baseline (speedup 1.0000x reference)
"""Trainium2 Bass kernel v4 for segment-causal GQA attention.

Sharding: 8 cores = batch (2) x kv-head (4); host sums the 4 row-parallel
Wo partial outputs per batch.  All device compute in fp16 (1 PE cycle/row
at any moving width, 2-byte DVE fast modes, half the DMA bytes of fp32).

Layout per core (T=1024, D=2048, H=128, G=4 q-heads):
  xt      [128, 16*1024]  x[b]^T d-tiles side by side
  qh[g]   [128, T]   rope'd, rstd-scaled q per head (transposed)
  kTn     [128, T]   rope'd k, with SCALE*rstd_k folded in per-column
  V       [128, 8*128]  v in [s,h] layout per 128-s-block

v4 structure (vs the v2 baseline):
  - rms sumsq via Pool partition_all_reduce on 64-partition halves
    (replaces the sel65/ones PE matmuls AND the partition_broadcasts:
    the all-reduce output is already broadcast).  The gpsimd reduce only
    works from base partition 0, so the upper half is copied down first.
  - denominator: accumulate masked P blocks on DVE, ONE
    partition_all_reduce per t-block instead of one per (t,s) block
  - qkv contraction as a single 512-wide matmul per s-block (one PSUM
    bank) and the 1/den normalize as a single 512-wide TT
  - DMA order: xt stream has priority; first x tile split so the PE
    starts at ~3.4us
  - chunk-1 post-processing (q rope for pair B, k rope/fold) is emitted
    as DVE "filler" ops popped inside the attention loop, and the v
    chunk-1 projection matmuls are PE fillers: attention over the
    chunk-0 t-blocks starts ~7us earlier than a strict phase split
  - all phase-1 activations (copies+sqrt) precede the single table
    switch to the exp set (forced by a dummy exp on the last sqrt)
"""

import sys

sys.path.insert(0, "/opt/trn_rl_repo")

import numpy as np

import concourse.bacc as bacc
import concourse.bass as bass  # noqa: F401
import concourse.tile as tile
from concourse import mybir
from concourse.bass_utils import run_bass_kernel_spmd

B, T, D = 2, 1024, 2048
N, K, H = 16, 4, 128
G = N // K
EPS = 1e-6
SCALE = H ** -0.5
ROPE_BASE = 10000.0
NCHUNK = 2
CW = T // NCHUNK        # 512
NTB = T // 128          # 8 t-blocks (and s-blocks)
ND = D // 128           # 16
F32 = mybir.dt.float32
F16 = mybir.dt.float16
MULT = mybir.AluOpType.mult
EXPB = -4.0             # exp bias: keeps P in fp16 range without max-sub

LAST_RESULTS = None


def _positions(seg):
    t = seg.shape[0]
    idx = np.arange(t, dtype=np.int64)
    is_start = np.concatenate([[True], seg[1:] != seg[:-1]])
    seg_start = np.maximum.accumulate(np.where(is_start, idx, 0))
    return (idx - seg_start).astype(np.float64)


def _classify(seg_rows):
    """Union-over-batches 128x128 block plan.

    Returns (plan, full, masks): plan[tb] = list of valid s-block indices;
    full[tb] = list of bools (all-ones mask in every batch -> skip the
    mask multiply); masks[b] = fp16 [128, n_partial*512] 0/1 pack of the
    partial blocks in plan order.
    """
    idx = np.arange(T)
    valids = []
    for b in range(B):
        seg = seg_rows[b]
        valids.append((seg[:, None] == seg[None, :]) & (idx[:, None] <= idx[None, :]))
    plan = []
    full = []
    packs = [[] for _ in range(B)]
    for tb in range(NTB):
        t0 = tb * 128
        ent = []
        fent = []
        for si in range(NTB):
            s0 = si * 128
            subs = [v[s0:s0 + 128, t0:t0 + 128] for v in valids]
            if any(s.any() for s in subs):
                ent.append(si)
                isfull = all(s.all() for s in subs)
                fent.append(isfull)
                if not isfull:
                    for b in range(B):
                        packs[b].append(subs[b])
        plan.append(ent)
        full.append(fent)
    masks = []
    for b in range(B):
        if packs[b]:
            m = np.concatenate([np.tile(p, (1, 4)) for p in packs[b]], axis=1)
        else:
            m = np.zeros((128, 512), bool)
        masks.append(np.ascontiguousarray(m.astype(np.float16)))
    return plan, full, masks


def _build_nc(plan, full, n_mask_cols):
    from contextlib import ExitStack

    nc = bacc.Bacc(None, target_bir_lowering=False, debug=False)
    xT_d = nc.dram_tensor("xT", [ND, 128, T], F16, kind="ExternalInput")
    wq_d = nc.dram_tensor("wq", [G, 128, ND * 128], F16, kind="ExternalInput")
    wk_d = nc.dram_tensor("wk", [128, ND * 128], F16, kind="ExternalInput")
    wv_d = nc.dram_tensor("wv", [128, ND * 128], F16, kind="ExternalInput")
    wo_d = nc.dram_tensor("wo", [G, 128, D], F16, kind="ExternalInput")
    # prescaled rope tables: cos/sin x per-partition rms-scale columns
    cqa_d = nc.dram_tensor("cqa", [128, T], F16, kind="ExternalInput")
    sqa_d = nc.dram_tensor("sqa", [128, T], F16, kind="ExternalInput")
    cqb_d = nc.dram_tensor("cqb", [128, T], F16, kind="ExternalInput")
    sqb_d = nc.dram_tensor("sqb", [128, T], F16, kind="ExternalInput")
    ckt_d = nc.dram_tensor("ckt", [128, T], F16, kind="ExternalInput")
    skt_d = nc.dram_tensor("skt", [128, T], F16, kind="ExternalInput")
    tblf_d = nc.dram_tensor("tblf", [128, 4], F32, kind="ExternalInput")
    tblh_d = nc.dram_tensor("tblh", [128, 128], F16, kind="ExternalInput")
    msk_d = nc.dram_tensor("masks", [128, n_mask_cols], F16, kind="ExternalInput")
    out_d = nc.dram_tensor("out", [T, D], F16, kind="ExternalOutput")

    from concourse import bass_isa
    RADD = bass_isa.ReduceOp.add

    es = ExitStack()
    with es:
        es.enter_context(nc.allow_low_precision("fp16 kernel"))
        tc = es.enter_context(tile.TileContext(nc))
        pool = lambda *a, **k: es.enter_context(tc.tile_pool(*a, **k))
        pp = pool(name="persist", bufs=1)

        # ---------------- persistent tiles ----------------
        xt = pp.tile([128, ND * T], F16, tag="xt", name="xt")  # 4MB
        # per-chunk tiles: chunk-1 writes (deferred into the attention
        # phase) must not create false deps on chunk-0 reads
        qhps = [pp.tile([128, NTB * G * 64], F16, tag=f"qhp{c}",
                        name=f"qhp{c}") for c in range(NCHUNK)]
        qhvs = [qhps[c][:].rearrange("p (a g t) -> p a g t", a=NTB // 2, g=G)
                for c in range(NCHUNK)]
        kTns = [pp.tile([128, CW], F16, tag=f"kTn{c}", name=f"kTn{c}")
                for c in range(NCHUNK)]
        Vs = [pp.tile([128, CW], F16, tag=f"V{c}", name=f"V{c}")
              for c in range(NCHUNK)]
        wqs = [pp.tile([128, ND * 128], F16, tag=f"wq{g}", name=f"wq{g}")
               for g in range(G)]
        wk_sb = pp.tile([128, ND * 128], F16, tag="wk", name="wk")
        wv_sb = pp.tile([128, ND * 128], F16, tag="wv", name="wv")
        wo_sb = [pp.tile([128, D], F16, tag=f"wo{g}", name=f"wo{g}")
                 for g in range(G)]
        cqa = pp.tile([128, T], F16, tag="cqa", name="cqa")
        sqa = pp.tile([128, T], F16, tag="sqa", name="sqa")
        cqb = pp.tile([128, T], F16, tag="cqb", name="cqb")
        sqb = pp.tile([128, T], F16, tag="sqb", name="sqb")
        ckt = pp.tile([128, T], F16, tag="ckt", name="ckt")
        skt = pp.tile([128, T], F16, tag="skt", name="skt")
        tblf = pp.tile([128, 4], F32, tag="tblf", name="tblf")
        tblh = pp.tile([128, 128], F16, tag="tblh", name="tblh")
        msk = pp.tile([128, n_mask_cols], F16, tag="msk", name="msk")
        vt0 = pp.tile([128, CW], F16, tag="vt0", name="vt0")
        vt1 = pp.tile([128, CW], F16, tag="vt1", name="vt1")

        kbias = tblf[:, 0:1]     # H*EPS
        qbias = tblf[:, 1:2]     # EPS
        expb = tblf[:, 2:3]      # exp bias column (EXPB)
        zcol = tblf[:, 3:4]      # 0.0
        iden = tblh[:, 0:128]    # fp16 identity

        # ---------------- DMA issue (consume order) ----------------
        def xt_ap(d):
            return xt[:, d * T:(d + 1) * T]

        xtv = xt[:].rearrange("p (a t) -> p a t", a=ND)
        # startup: tiny first tiles (first matmul needs wq0 d0-cols and
        # xt d0 chunk-0 only); then the xt stream gets absolute priority
        # (pair-A consumes one 2-tile DMA per ~1.7us of matmul); weight
        # tails are split so their transfers never delay the xt stream
        # past its consumption point.
        nc.sync.dma_start(wqs[0][:, 0:256], wq_d[0][:, 0:256])
        nc.sync.dma_start(xtv[:, 0, 0:CW], xT_d[0][:, 0:CW])
        nc.sync.dma_start(wqs[2][:, 0:256], wq_d[2][:, 0:256])
        nc.sync.dma_start(xtv[:, 0, CW:2 * CW], xT_d[0][:, CW:2 * CW])
        nc.sync.dma_start(xtv[:, 1:2, :], xT_d[1:2].transpose([1, 0, 2]))
        nc.sync.dma_start(xtv[:, 2:4, :], xT_d[2:4].transpose([1, 0, 2]))
        nc.sync.dma_start(wqs[0][:, 256:768], wq_d[0][:, 256:768])
        nc.sync.dma_start(wqs[2][:, 256:768], wq_d[2][:, 256:768])
        nc.sync.dma_start(xtv[:, 4:6, :], xT_d[4:6].transpose([1, 0, 2]))
        nc.sync.dma_start(xtv[:, 6:8, :], xT_d[6:8].transpose([1, 0, 2]))
        nc.sync.dma_start(wqs[0][:, 768:1280], wq_d[0][:, 768:1280])
        nc.sync.dma_start(wqs[2][:, 768:1280], wq_d[2][:, 768:1280])
        nc.sync.dma_start(xtv[:, 8:10, :], xT_d[8:10].transpose([1, 0, 2]))
        nc.sync.dma_start(wqs[0][:, 1280:2048], wq_d[0][:, 1280:2048])
        nc.sync.dma_start(wqs[2][:, 1280:2048], wq_d[2][:, 1280:2048])
        nc.sync.dma_start(xtv[:, 10:12, :], xT_d[10:12].transpose([1, 0, 2]))
        nc.sync.dma_start(xtv[:, 12:14, :], xT_d[12:14].transpose([1, 0, 2]))
        nc.sync.dma_start(xtv[:, 14:16, :], xT_d[14:16].transpose([1, 0, 2]))
        nc.sync.dma_start(tblf[:], tblf_d[:])
        nc.sync.dma_start(wqs[1][:], wq_d[1])
        nc.sync.dma_start(wqs[3][:], wq_d[3])
        nc.sync.dma_start(cqa[:], cqa_d[:])
        nc.sync.dma_start(sqa[:], sqa_d[:])
        nc.sync.dma_start(cqb[:], cqb_d[:])
        nc.sync.dma_start(sqb[:], sqb_d[:])
        nc.sync.dma_start(wv_sb[:], wv_d[:])
        nc.sync.dma_start(ckt[:], ckt_d[:])
        nc.sync.dma_start(skt[:], skt_d[:])
        nc.sync.dma_start(wk_sb[:], wk_d[:])
        nc.sync.dma_start(tblh[:], tblh_d[:])
        nc.sync.dma_start(msk[:], msk_d[:])
        for g in range(G):
            nc.sync.dma_start(wo_sb[g][:], wo_d[g])

        # ---------------- phase-1 pools ----------------
        # (SBUF pools stay open through attention: the deferred chunk-1
        # DVE filler ops allocate their temporaries from them)
        sbs = pool(name="sb_stream", bufs=2)
        rsp = pool(name="ropes", bufs=2)
        ps1 = ExitStack()
        psproj = ps1.enter_context(tc.tile_pool(name="ps_proj", bufs=7, space="PSUM"))
        ps_v = ps1.enter_context(tc.tile_pool(name="ps_v", bufs=1, space="PSUM"))

        def project4(wa, wb):
            """d-outer accumulation: psums[(fi, c)] = [128, CW] f32."""
            pss = {(fi, c): psproj.tile([128, CW], F32, tag="proj", name="proj")
                   for fi in range(2) for c in range(NCHUNK)}
            for d_i in range(ND):
                for c in range(NCHUNK):
                    for fi, w in enumerate((wa, wb)):
                        nc.tensor.matmul(
                            pss[(fi, c)][:],
                            w[:, d_i * 128:(d_i + 1) * 128],
                            xt_ap(d_i)[:, c * CW:(c + 1) * CW],
                            start=(d_i == 0), stop=(d_i == ND - 1))
            return pss

        def q_copies(pss):
            pcs = {}
            for c in range(NCHUNK):
                pca = sbs.tile([128, CW], F16, tag="pca", name="pca", bufs=4)
                pcb = sbs.tile([128, CW], F16, tag="pcb", name="pcb", bufs=4)
                nc.scalar.copy(pca[:], pss[(0, c)][:])
                nc.scalar.copy(pcb[:], pss[(1, c)][:])
                pcs[c] = (pca, pcb)
            return pcs

        def q_prefix(pca, pcb):
            """sumsq halves -> Pool all-reduce -> Act sqrt.

            Head ga lives in partitions 0:64 of BOTH pca and pcb; gb in
            64:128.  The gpsimd all-reduce only works from base partition
            0, so the gb half is copied down first.
            """
            sqt = sbs.tile([128, CW], F16, tag="sq", name="sq")
            ssum = sbs.tile([128, CW], F16, tag="ssum", name="ssum")
            nc.vector.tensor_mul(sqt[:], pca[:], pca[:])
            nc.vector.tensor_mul(ssum[:], pcb[:], pcb[:])
            nc.vector.tensor_add(ssum[:], ssum[:], sqt[:])
            shi = sbs.tile([64, CW], F16, tag="shi", name="shi")
            nc.vector.tensor_copy(shi[:], ssum[64:128, :])
            ssqa = sbs.tile([64, CW], F16, tag="ssqa", name="ssqa")
            ssqb = sbs.tile([64, CW], F16, tag="ssqb", name="ssqb")
            nc.gpsimd.partition_all_reduce(ssqa[:], ssum[0:64, :],
                                           channels=64, reduce_op=RADD)
            nc.gpsimd.partition_all_reduce(ssqb[:], shi[:],
                                           channels=64, reduce_op=RADD)
            sra = sbs.tile([64, CW], F16, tag="sra", name="sra", bufs=3)
            srb = sbs.tile([64, CW], F16, tag="srb", name="srb", bufs=3)
            nc.scalar.activation(sra[:], ssqa[:],
                                 mybir.ActivationFunctionType.Sqrt,
                                 bias=qbias[0:64, :], scale=float(1.0 / H))
            nc.scalar.activation(srb[:], ssqb[:],
                                 mybir.ActivationFunctionType.Sqrt,
                                 bias=qbias[0:64, :], scale=float(1.0 / H))
            return sra, srb

        def q_rope_ops(pca, pcb, sra, srb, ga, gb, c, pool_muls=False):
            """DVE tail of the q post-chain, as single-op closures:
            rope (independent of the rstd round-trip, so it leads),
            reciprocals, base-64 lift, and the rstd multiplies.
            pool_muls moves the gb-half multiplies to the Pool engine --
            only worth it for the deferred chains popped while the DVE
            is saturated with attention work."""
            gb_eng = nc.gpsimd if pool_muls else nc.vector
            cs = slice(c * CW, (c + 1) * CW)
            st = {}

            def f1():
                st["m1"] = sbs.tile([128, CW], F16, tag="m1", name="m1")
                nc.vector.tensor_mul(st["m1"][:], pca[:], cqa[:, cs])

            def f2():
                st["m2"] = sbs.tile([128, CW], F16, tag="m2", name="m2")
                nc.vector.tensor_mul(st["m2"][:], pcb[:], sqb[:, cs])

            def f3():
                st["ra"] = rsp.tile([128, CW], F16, tag="ra", name="ra")
                nc.vector.tensor_sub(st["ra"][:], st["m1"][:], st["m2"][:])

            def f4():
                nc.vector.tensor_mul(st["m1"][:], pcb[:], cqb[:, cs])

            def f5():
                nc.vector.tensor_mul(st["m2"][:], pca[:], sqa[:, cs])

            def f6():
                st["rb"] = rsp.tile([128, CW], F16, tag="rb", name="rb")
                nc.vector.tensor_add(st["rb"][:], st["m1"][:], st["m2"][:])

            def f7():
                st["rsta"] = sbs.tile([64, CW], F16, tag="rsta", name="rsta")
                nc.vector.reciprocal(st["rsta"][:], sra[:])

            def f8():
                st["rstb"] = sbs.tile([64, CW], F16, tag="rstb", name="rstb")
                nc.vector.reciprocal(st["rstb"][:], srb[:])

            def f9():
                st["rsthi"] = sbs.tile([128, CW], F16, tag="rsthi",
                                       name="rsthi")
                nc.vector.tensor_copy(st["rsthi"][64:128, :], st["rstb"][:])

            qhv = qhvs[c]
            tbs = slice(0, 4)
            r3 = lambda ap: ap.rearrange("p (a t) -> p a t", a=4)

            def f10():
                nc.vector.tensor_mul(qhv[0:64, tbs, ga, :],
                                     r3(st["ra"][0:64, :]),
                                     r3(st["rsta"][:]))

            def f11():
                nc.vector.tensor_mul(qhv[64:128, tbs, ga, :],
                                     r3(st["rb"][0:64, :]),
                                     r3(st["rsta"][:]))

            def f12():
                gb_eng.tensor_mul(qhv[0:64, tbs, gb, :],
                                  r3(st["ra"][64:128, :]),
                                  r3(st["rsthi"][64:128, :]))

            def f13():
                gb_eng.tensor_mul(qhv[64:128, tbs, gb, :],
                                  r3(st["rb"][64:128, :]),
                                  r3(st["rsthi"][64:128, :]))

            return [f1, f2, f3, f4, f5, f6, f7, f8, f9, f10, f11, f12, f13]

        def kv_proj(w, c):
            psk = psproj.tile([128, CW], F32, tag="proj", name="proj")
            for d_i in range(ND):
                nc.tensor.matmul(psk[:], w[:, d_i * 128:(d_i + 1) * 128],
                                 xt_ap(d_i)[:, c * CW:(c + 1) * CW],
                                 start=(d_i == 0), stop=(d_i == ND - 1))
            return psk

        def k_prefix(pck):
            sqk = sbs.tile([128, CW], F16, tag="sqk", name="sqk")
            nc.vector.tensor_mul(sqk[:], pck[:], pck[:])
            kssr = sbs.tile([128, CW], F16, tag="kssr", name="kssr")
            nc.gpsimd.partition_all_reduce(kssr[:], sqk[:],
                                           channels=128, reduce_op=RADD)
            ksq = sbs.tile([128, CW], F16, tag="ksq", name="ksq")
            nc.scalar.activation(ksq[:], kssr[:],
                                 mybir.ActivationFunctionType.Sqrt,
                                 bias=kbias, scale=1.0)
            return ksq

        def k_rope_ops(pck, ksq, c, pool_muls=False):
            cs = slice(c * CW, (c + 1) * CW)
            kT = kTns[c]
            eng = nc.gpsimd if pool_muls else nc.vector
            st = {}
            k0, k1 = pck[0:64, :], pck[64:128, :]

            def f1():
                st["m1"] = sbs.tile([128, CW], F16, tag="km1", name="km1")
                eng.tensor_mul(st["m1"][0:64, :], k0, ckt[0:64, cs])

            def f2():
                st["m2"] = sbs.tile([128, CW], F16, tag="km2", name="km2")
                eng.tensor_mul(st["m2"][0:64, :], k1, skt[64:128, cs])

            def f3():
                nc.vector.tensor_sub(kT[0:64, :], st["m1"][0:64, :],
                                     st["m2"][0:64, :])

            def f4():
                st["m3"] = sbs.tile([128, CW], F16, tag="km3", name="km3")
                eng.tensor_mul(st["m3"][0:64, :], k1, ckt[64:128, cs])

            def f5():
                st["m4"] = sbs.tile([128, CW], F16, tag="km4", name="km4")
                eng.tensor_mul(st["m4"][0:64, :], k0, skt[0:64, cs])

            def f6():
                nc.vector.tensor_add(kT[64:128, :], st["m3"][0:64, :],
                                     st["m4"][0:64, :])

            def f7():
                st["krst"] = sbs.tile([128, CW], F16, tag="krst", name="krst")
                nc.vector.reciprocal(st["krst"][:], ksq[:])

            def f8():
                nc.vector.tensor_mul(kT[:], kT[:], st["krst"][:])

            return [f1, f2, f3, f4, f5, f6, f7, f8]

        # ---------------- q pair A ----------------
        pssA = project4(wqs[0], wqs[2])
        pcsA = q_copies(pssA)
        sraA0, srbA0 = q_prefix(*pcsA[0])
        for f in q_rope_ops(*pcsA[0], sraA0, srbA0, 0, 1, 0):
            f()
        sraA1, srbA1 = q_prefix(*pcsA[1])
        for f in q_rope_ops(*pcsA[1], sraA1, srbA1, 0, 1, 1):
            f()

        # ---------------- q pair B ----------------
        pssB = project4(wqs[1], wqs[3])
        pcsB = q_copies(pssB)
        sraB0, srbB0 = q_prefix(*pcsB[0])
        for f in q_rope_ops(*pcsB[0], sraB0, srbB0, 2, 3, 0):
            f()

        # ---------------- k chunk 1, then k chunk 0, then v chunk 0 ----
        # ksq1 is a sqrt-set activation and the warm exp (table switch)
        # must follow ALL sqrts: k-c1 projects first so its sqrt chain
        # finishes early (kTn1 itself is needed much later); k-c0 next
        # (its fold gates the first logits); v-c0 last (fastest post,
        # done on DVE so it does not queue behind the Act sqrts).
        psk1 = kv_proj(wk_sb, 1)
        pck1 = sbs.tile([128, CW], F16, tag="pck", name="pck")
        nc.scalar.copy(pck1[:], psk1[:])
        ksq1 = k_prefix(pck1)
        psk0 = kv_proj(wk_sb, 0)
        pck0 = sbs.tile([128, CW], F16, tag="pck", name="pck")
        nc.scalar.copy(pck0[:], psk0[:])
        ksq0 = k_prefix(pck0)
        for f in k_rope_ops(pck0, ksq0, 0):
            f()
        sraB1, srbB1 = q_prefix(*pcsB[1])   # rope for B-c1 deferred
        # k-c1 rope/fold deferred to the attention-phase DVE fillers

        psv0 = kv_proj(wv_sb, 0)
        nc.vector.tensor_copy(vt0[:], psv0[:])
        for jj in range(4):
            vp = ps_v.tile([128, 128], F16, tag="pv", name="pv")
            nc.tensor.transpose(vp[:], vt0[:, jj * 128:(jj + 1) * 128], iden)
            nc.vector.tensor_copy(Vs[0][:, jj * 128:(jj + 1) * 128], vp[:])

        # switch the Act table to the exp set now: every sqrt-set
        # activation has been emitted above; reading the last k sqrt
        # output places this after all of them
        warm = sbs.tile([1, 2], F16, tag="warm", name="warm")
        nc.scalar.activation(warm[:], ksq1[0:1, 0:2],
                             mybir.ActivationFunctionType.Exp,
                             bias=zcol[0:1, :], scale=-1.0)

        # deferred chunk-1 DVE work, popped between attention ops
        dve_fillers = (q_rope_ops(*pcsB[1], sraB1, srbB1, 2, 3, 1,
                                    pool_muls=True)
                       + k_rope_ops(pck1, ksq1, 1, pool_muls=True))
        dstate = [0]

        def pop_dve(n):
            while n > 0 and dstate[0] < len(dve_fillers):
                dve_fillers[dstate[0]]()
                dstate[0] += 1
                n -= 1

        ps1.close()

        # ---------------- attention + out-projection per t-block ----------------
        sbP = pool(name="sbP", bufs=5)
        sbD = pool(name="sbD", bufs=2)
        sbx = pool(name="sbx", bufs=3)
        osp = pool(name="outs", bufs=3)
        ps_lg = pool(name="ps_lg", bufs=3, space="PSUM")
        ps_qkv = pool(name="ps_qkv", bufs=1, space="PSUM")
        ps_op = pool(name="ps_op", bufs=2, space="PSUM")
        ps_vc1 = pool(name="ps_vc1", bufs=1, space="PSUM")
        ps_pv = pool(name="ps_pv", bufs=1, space="PSUM")

        # deferred v chunk-1 projection: 16 matmuls used as PE filler
        # between the first softmax chains (their latency would otherwise
        # idle the PE before any out-projection work exists)
        psv1 = ps_vc1.tile([128, CW], F32, tag="vc1", name="vc1")
        pe_fillers = []
        for d_i in range(ND):
            def _mk(d):
                def f():
                    nc.tensor.matmul(
                        psv1[:], wv_sb[:, d * 128:(d + 1) * 128],
                        xt_ap(d)[:, CW:2 * CW],
                        start=(d == 0), stop=(d == ND - 1))
                return f
            pe_fillers.append(_mk(d_i))

        def _vt1_copy():
            nc.scalar.copy(vt1[:], psv1[:])
        pe_fillers.append(_vt1_copy)
        fstate = [0]

        def pop_fill(n):
            while n > 0 and fstate[0] < len(pe_fillers):
                pe_fillers[fstate[0]]()
                fstate[0] += 1
                n -= 1

        def v_c1_transposes():
            pop_fill(len(pe_fillers))
            pop_dve(len(dve_fillers))
            for jj in range(4):
                vp = ps_pv.tile([128, 128], F16, tag="pv1", name="pv1")
                nc.tensor.transpose(vp[:], vt1[:, jj * 128:(jj + 1) * 128],
                                    iden)
                nc.scalar.copy(Vs[1][:, jj * 128:(jj + 1) * 128], vp[:])

        # mask-column offset of each (tb, i) partial block, in pack order
        moff = {}
        off = 0
        for tb in range(NTB):
            for i in range(len(plan[tb])):
                if not full[tb][i]:
                    moff[(tb, i)] = off
                    off += 1

        def outproj_dc(tb, qkvh, ob, dc, flush):
            t0 = tb * 128
            op = ps_op.tile([128, CW], F32, tag="op", name="op")
            for g in range(G):
                nc.tensor.matmul(op[:],
                                 qkvh[:, g * 128:(g + 1) * 128],
                                 wo_sb[g][:, dc * CW:(dc + 1) * CW],
                                 start=(g == 0), stop=(g == G - 1))
            if dc % 2 == 0:
                nc.vector.tensor_copy(ob[:, dc * CW:(dc + 1) * CW], op[:])
            else:
                nc.scalar.copy(ob[:, dc * CW:(dc + 1) * CW], op[:])
            if flush:
                nc.sync.dma_start(
                    out_d[t0:t0 + 128, dc * CW:(dc + 1) * CW],
                    ob[:, dc * CW:(dc + 1) * CW])
            elif dc == 3:
                nc.sync.dma_start(out_d[t0:t0 + 128, :], ob[:])

        # pending out-projection units: each is one (tb, dc) 4-matmul
        # group; keeping up to one tb's worth pending lets outproj work
        # fill softmax-chain latency across TWO later t-blocks
        pending = []
        flush_tbs = set()

        def emit_pending(n):
            while n > 0 and pending:
                tb_, qk_, ob_, dc_ = pending.pop(0)
                outproj_dc(tb_, qk_, ob_, dc_, tb_ in flush_tbs)
                n -= 1

        # chunk-0-only t-blocks first (their logits/qkv need no chunk-1
        # k/V), biggest first within each group for outproj filler supply;
        # the smallest chunk-0 tb goes last (shortest tail den chain)
        c0_tbs = sorted((t for t in range(NTB) if max(plan[t]) < NTB // 2),
                        key=lambda t: -len(plan[t]))
        c1_tbs = sorted((t for t in range(NTB) if max(plan[t]) >= NTB // 2),
                        key=lambda t: -len(plan[t]))
        tail_tb = [c0_tbs.pop()] if len(c0_tbs) > 1 else []
        tb_order = c0_tbs + c1_tbs + tail_tb
        flush_tbs.update(tb_order[-2:])

        pop_fill(6)
        did_vc1 = False
        for tb in tb_order:
            ent = plan[tb]
            nv = len(ent)
            if not did_vc1 and max(ent) >= NTB // 2:
                v_c1_transposes()
                did_vc1 = True
            qkvps = ps_qkv.tile([128, 512], F32, tag="qkvp", name="qkvp")
            qkvh = sbx.tile([128, 512], F16, tag="qkvh", name="qkvh")
            ob = osp.tile([128, D], F16, tag="ob", name="ob")
            acc = (sbD.tile([128, 512], F16, tag="dacc", name="dacc")
                   if nv > 1 else None)

            Ps = {}
            for i, si in enumerate(ent):
                # logits for all 4 heads of this (tb, si) block
                lg = ps_lg.tile([128, 512], F32, tag="lg", name="lg")
                sc, sj = divmod(si, NTB // 2)
                tc_, tj = divmod(tb, NTB // 2)
                nc.tensor.matmul(lg[:], kTns[sc][:, sj * 128:(sj + 1) * 128],
                                 qhps[tc_][:, tj * 512:(tj + 1) * 512],
                                 start=True, stop=True)
                P = sbP.tile([128, 512], F16, tag="P", name="P")
                nc.scalar.activation(P[:], lg[:],
                                     mybir.ActivationFunctionType.Exp,
                                     bias=expb, scale=1.0)
                if (tb, i) in moff:
                    mo = moff[(tb, i)]
                    nc.vector.tensor_mul(P[:], P[:],
                                         msk[:, mo * 512:(mo + 1) * 512])
                Ps[i] = P
                if pending:
                    emit_pending(1)
                    pop_fill(1)
                else:
                    pop_fill(3)
                nc.tensor.matmul(qkvps[:], Vs[sc][:, sj * 128:(sj + 1) * 128],
                                 P[:], start=(i == 0), stop=(i == nv - 1))
                if i == 1:
                    nc.vector.tensor_add(acc[:], Ps[0][:], P[:])
                elif i > 1:
                    nc.vector.tensor_add(acc[:], acc[:], P[:])
                pop_dve(2)
            den_src = acc if nv > 1 else Ps[0]
            den = sbD.tile([128, 512], F16, tag="den", name="den")
            nc.gpsimd.partition_all_reduce(den[:], den_src[:],
                                           channels=128, reduce_op=RADD)
            rec = sbD.tile([128, 512], F16, tag="recg", name="recg")
            nc.vector.reciprocal(rec[:], den[:])
            nc.vector.tensor_mul(qkvh[:], qkvps[:], rec[:])
            pop_dve(1)
            Ps.clear()
            for dc in range(4):
                pending.append((tb, qkvh, ob, dc))
            while len(pending) > 8:
                emit_pending(1)
        emit_pending(len(pending))

    nc.finalize()
    return nc


_CACHE = {}


def kernel(x, segment_ids, Wq, Wk, Wv, Wo, q_scale, k_scale):
    global LAST_RESULTS
    import os

    x = np.asarray(x, np.float32)
    seg = np.asarray(segment_ids)
    Wq = np.asarray(Wq, np.float32)
    Wk = np.asarray(Wk, np.float32)
    Wv = np.asarray(Wv, np.float32)
    Wo = np.asarray(Wo, np.float32)
    q_scale = np.asarray(q_scale, np.float32)
    k_scale = np.asarray(k_scale, np.float32)

    plan, full, masks = _classify([seg[b] for b in range(B)])
    key = repr((plan, full))
    if key not in _CACHE:
        _CACHE[key] = _build_nc(plan, full, masks[0].shape[1])
    nc = _CACHE[key]

    half = H // 2
    timescale = ROPE_BASE ** (2.0 * np.arange(half, dtype=np.float64) / H)
    qscA = np.tile(q_scale[:64], 2).astype(np.float64)[:, None]
    qscB = np.tile(q_scale[64:], 2).astype(np.float64)[:, None]
    kvec = k_scale.astype(np.float64)[:, None]
    tabs = []  # per batch: (cqa, sqa, cqb, sqb, ckt, skt)
    for b in range(B):
        pos = _positions(seg[b])
        sinus = pos[:, None] / timescale[None, :]
        sT = np.sin(sinus).T
        cT = np.cos(sinus).T
        c2 = np.vstack([cT, cT])
        s2 = np.vstack([sT, sT])
        tabs.append(tuple(
            np.ascontiguousarray(a, np.float16)
            for a in (c2 * qscA, s2 * qscA, c2 * qscB, s2 * qscB,
                      c2 * kvec, s2 * kvec)))

    tblf = np.zeros((128, 4), np.float32)
    tblf[:, 0] = H * EPS
    tblf[:, 1] = EPS
    tblf[:, 2] = EXPB
    tblf[:, 3] = 0.0
    tblh = np.ascontiguousarray(np.eye(128, dtype=np.float16))

    in_maps = []
    for core in range(8):
        b, kv = core // K, core % K
        qcols = []
        for hv in range(2):
            for g4 in range(G):
                base = kv * 512 + g4 * 128 + hv * 64
                qcols.extend(range(base, base + 64))
        qp = np.array(qcols)
        wq_t = np.ascontiguousarray(
            Wq[:, qp].reshape(ND, 128, G, 128).transpose(2, 1, 0, 3)
            .reshape(G, 128, ND * 128), np.float16)
        wk_t = np.ascontiguousarray(
            Wk[:, kv * 128:(kv + 1) * 128].reshape(ND, 128, 128)
            .transpose(1, 0, 2).reshape(128, ND * 128), np.float16)
        wv_t = np.ascontiguousarray(
            Wv[:, kv * 128:(kv + 1) * 128].reshape(ND, 128, 128)
            .transpose(1, 0, 2).reshape(128, ND * 128), np.float16)
        wo_t = np.ascontiguousarray(
            Wo[kv * 512:(kv + 1) * 512].reshape(G, 128, D), np.float16)
        xt_t = np.ascontiguousarray(
            x[b].T.reshape(ND, 128, T), np.float16)
        cqa, sqa, cqb, sqb, ckt, skt = tabs[b]
        in_maps.append({
            "xT": xt_t, "wq": wq_t, "wk": wk_t, "wv": wv_t, "wo": wo_t,
            "cqa": cqa, "sqa": sqa, "cqb": cqb, "sqb": sqb,
            "ckt": ckt, "skt": skt,
            "tblf": tblf, "tblh": tblh, "masks": masks[b],
        })

    do_trace = os.environ.get("BASS_TRACE") == "1"
    res = run_bass_kernel_spmd(
        nc, in_maps, core_ids=list(range(8)), trace=do_trace)
    LAST_RESULTS = res

    out = np.zeros((B, T, D), np.float32)
    for core in range(8):
        out[core // K] += res.results[core]["out"].astype(np.float32)
    return out


# revision 37
# speedup vs baseline: 1.0101x; 1.0101x over previous
"""Trainium2 Bass kernel v4 for segment-causal GQA attention.

Sharding: 8 cores = batch (2) x kv-head (4); host sums the 4 row-parallel
Wo partial outputs per batch.  All device compute in fp16 (1 PE cycle/row
at any moving width, 2-byte DVE fast modes, half the DMA bytes of fp32).

Layout per core (T=1024, D=2048, H=128, G=4 q-heads):
  xt      [128, 16*1024]  x[b]^T d-tiles side by side
  qh[g]   [128, T]   rope'd, rstd-scaled q per head (transposed)
  kTn     [128, T]   rope'd k, with SCALE*rstd_k folded in per-column
  V       [128, 8*128]  v in [s,h] layout per 128-s-block

v4 structure (vs the v2 baseline):
  - rms sumsq via Pool partition_all_reduce on 64-partition halves
    (replaces the sel65/ones PE matmuls AND the partition_broadcasts:
    the all-reduce output is already broadcast).  The gpsimd reduce only
    works from base partition 0, so the upper half is copied down first.
  - denominator: accumulate masked P blocks on DVE, ONE
    partition_all_reduce per t-block instead of one per (t,s) block
  - qkv contraction as a single 512-wide matmul per s-block (one PSUM
    bank) and the 1/den normalize as a single 512-wide TT
  - DMA order: xt stream has priority; first x tile split so the PE
    starts at ~3.4us
  - chunk-1 post-processing (q rope for pair B, k rope/fold) is emitted
    as DVE "filler" ops popped inside the attention loop, and the v
    chunk-1 projection matmuls are PE fillers: attention over the
    chunk-0 t-blocks starts ~7us earlier than a strict phase split
  - all phase-1 activations (copies+sqrt) precede the single table
    switch to the exp set (forced by a dummy exp on the last sqrt)
"""

import sys

sys.path.insert(0, "/opt/trn_rl_repo")

import numpy as np

import concourse.bacc as bacc
import concourse.bass as bass  # noqa: F401
import concourse.tile as tile
from concourse import mybir
from concourse.bass_utils import run_bass_kernel_spmd

B, T, D = 2, 1024, 2048
N, K, H = 16, 4, 128
G = N // K
EPS = 1e-6
SCALE = H ** -0.5
ROPE_BASE = 10000.0
NCHUNK = 2
CW = T // NCHUNK        # 512
NTB = T // 128          # 8 t-blocks (and s-blocks)
ND = D // 128           # 16
F32 = mybir.dt.float32
F16 = mybir.dt.float16
MULT = mybir.AluOpType.mult
EXPB = -4.0             # exp bias: keeps P in fp16 range without max-sub

LAST_RESULTS = None


def _positions(seg):
    t = seg.shape[0]
    idx = np.arange(t, dtype=np.int64)
    is_start = np.concatenate([[True], seg[1:] != seg[:-1]])
    seg_start = np.maximum.accumulate(np.where(is_start, idx, 0))
    return (idx - seg_start).astype(np.float64)


def _classify(seg_rows):
    """Union-over-batches 128x128 block plan.

    Returns (plan, full, masks): plan[tb] = list of valid s-block indices;
    full[tb] = list of bools (all-ones mask in every batch -> skip the
    mask multiply); masks[b] = fp16 [128, n_partial*512] 0/1 pack of the
    partial blocks in plan order.
    """
    idx = np.arange(T)
    valids = []
    for b in range(B):
        seg = seg_rows[b]
        valids.append((seg[:, None] == seg[None, :]) & (idx[:, None] <= idx[None, :]))
    plan = []
    full = []
    packs = [[] for _ in range(B)]
    for tb in range(NTB):
        t0 = tb * 128
        ent = []
        fent = []
        for si in range(NTB):
            s0 = si * 128
            subs = [v[s0:s0 + 128, t0:t0 + 128] for v in valids]
            if any(s.any() for s in subs):
                ent.append(si)
                isfull = all(s.all() for s in subs)
                fent.append(isfull)
                if not isfull:
                    for b in range(B):
                        packs[b].append(subs[b])
        plan.append(ent)
        full.append(fent)
    masks = []
    for b in range(B):
        if packs[b]:
            m = np.concatenate([np.tile(p, (1, 4)) for p in packs[b]], axis=1)
        else:
            m = np.zeros((128, 512), bool)
        masks.append(np.ascontiguousarray(m.astype(np.float16)))
    return plan, full, masks


def _build_nc(plan, full, n_mask_cols):
    from contextlib import ExitStack

    nc = bacc.Bacc(None, target_bir_lowering=False, debug=False)
    xT_d = nc.dram_tensor("xT", [ND, 128, T], F16, kind="ExternalInput")
    wq_d = nc.dram_tensor("wq", [G, 128, ND * 128], F16, kind="ExternalInput")
    wk_d = nc.dram_tensor("wk", [128, ND * 128], F16, kind="ExternalInput")
    wv_d = nc.dram_tensor("wv", [128, ND * 128], F16, kind="ExternalInput")
    wo_d = nc.dram_tensor("wo", [G, 128, D], F16, kind="ExternalInput")
    # prescaled rope tables: cos/sin x per-partition rms-scale columns
    cqa_d = nc.dram_tensor("cqa", [128, T], F16, kind="ExternalInput")
    sqa_d = nc.dram_tensor("sqa", [128, T], F16, kind="ExternalInput")
    cqb_d = nc.dram_tensor("cqb", [128, T], F16, kind="ExternalInput")
    sqb_d = nc.dram_tensor("sqb", [128, T], F16, kind="ExternalInput")
    ckt_d = nc.dram_tensor("ckt", [128, T], F16, kind="ExternalInput")
    skt_d = nc.dram_tensor("skt", [128, T], F16, kind="ExternalInput")
    tblf_d = nc.dram_tensor("tblf", [128, 4], F32, kind="ExternalInput")
    tblh_d = nc.dram_tensor("tblh", [128, 128], F16, kind="ExternalInput")
    msk_d = nc.dram_tensor("masks", [128, n_mask_cols], F16, kind="ExternalInput")
    out_d = nc.dram_tensor("out", [T, D], F16, kind="ExternalOutput")

    from concourse import bass_isa
    RADD = bass_isa.ReduceOp.add

    es = ExitStack()
    with es:
        es.enter_context(nc.allow_low_precision("fp16 kernel"))
        tc = es.enter_context(tile.TileContext(nc))
        pool = lambda *a, **k: es.enter_context(tc.tile_pool(*a, **k))
        pp = pool(name="persist", bufs=1)

        # ---------------- persistent tiles ----------------
        xt = pp.tile([128, ND * T], F16, tag="xt", name="xt")  # 4MB
        # per-chunk tiles: chunk-1 writes (deferred into the attention
        # phase) must not create false deps on chunk-0 reads
        qhps = [pp.tile([128, NTB * G * 64], F16, tag=f"qhp{c}",
                        name=f"qhp{c}") for c in range(NCHUNK)]
        qhvs = [qhps[c][:].rearrange("p (a g t) -> p a g t", a=NTB // 2, g=G)
                for c in range(NCHUNK)]
        kTns = [pp.tile([128, CW], F16, tag=f"kTn{c}", name=f"kTn{c}")
                for c in range(NCHUNK)]
        Vs = [pp.tile([128, CW], F16, tag=f"V{c}", name=f"V{c}")
              for c in range(NCHUNK)]
        wqs = [pp.tile([128, ND * 128], F16, tag=f"wq{g}", name=f"wq{g}")
               for g in range(G)]
        wk_sb = pp.tile([128, ND * 128], F16, tag="wk", name="wk")
        wv_sb = pp.tile([128, ND * 128], F16, tag="wv", name="wv")
        wo_sb = [pp.tile([128, D], F16, tag=f"wo{g}", name=f"wo{g}")
                 for g in range(G)]
        cqa = pp.tile([128, T], F16, tag="cqa", name="cqa")
        sqa = pp.tile([128, T], F16, tag="sqa", name="sqa")
        cqb = pp.tile([128, T], F16, tag="cqb", name="cqb")
        sqb = pp.tile([128, T], F16, tag="sqb", name="sqb")
        ckt = pp.tile([128, T], F16, tag="ckt", name="ckt")
        skt = pp.tile([128, T], F16, tag="skt", name="skt")
        tblf = pp.tile([128, 4], F32, tag="tblf", name="tblf")
        tblh = pp.tile([128, 128], F16, tag="tblh", name="tblh")
        msk = pp.tile([128, n_mask_cols], F16, tag="msk", name="msk")
        vt0 = pp.tile([128, CW], F16, tag="vt0", name="vt0")
        vt1 = pp.tile([128, CW], F16, tag="vt1", name="vt1")

        kbias = tblf[:, 0:1]     # H*EPS
        qbias = tblf[:, 1:2]     # EPS
        expb = tblf[:, 2:3]      # exp bias column (EXPB)
        zcol = tblf[:, 3:4]      # 0.0
        iden = tblh[:, 0:128]    # fp16 identity

        # ---------------- DMA issue (consume order) ----------------
        def xt_ap(d):
            return xt[:, d * T:(d + 1) * T]

        xtv = xt[:].rearrange("p (a t) -> p a t", a=ND)
        # startup: tiny first tiles (first matmul needs wq0 d0-cols and
        # xt d0 chunk-0 only); then the xt stream gets absolute priority
        # (pair-A consumes one 2-tile DMA per ~1.7us of matmul); weight
        # tails are split so their transfers never delay the xt stream
        # past its consumption point.
        nc.sync.dma_start(wqs[0][:, 0:256], wq_d[0][:, 0:256])
        nc.sync.dma_start(xtv[:, 0, 0:CW], xT_d[0][:, 0:CW])
        nc.sync.dma_start(wqs[2][:, 0:256], wq_d[2][:, 0:256])
        nc.sync.dma_start(xtv[:, 0, CW:2 * CW], xT_d[0][:, CW:2 * CW])
        nc.sync.dma_start(xtv[:, 1:2, :], xT_d[1:2].transpose([1, 0, 2]))
        nc.sync.dma_start(xtv[:, 2:4, :], xT_d[2:4].transpose([1, 0, 2]))
        nc.sync.dma_start(wqs[0][:, 256:768], wq_d[0][:, 256:768])
        nc.sync.dma_start(wqs[2][:, 256:768], wq_d[2][:, 256:768])
        nc.sync.dma_start(xtv[:, 4:6, :], xT_d[4:6].transpose([1, 0, 2]))
        nc.sync.dma_start(xtv[:, 6:8, :], xT_d[6:8].transpose([1, 0, 2]))
        nc.sync.dma_start(wqs[0][:, 768:1280], wq_d[0][:, 768:1280])
        nc.sync.dma_start(wqs[2][:, 768:1280], wq_d[2][:, 768:1280])
        nc.sync.dma_start(xtv[:, 8:10, :], xT_d[8:10].transpose([1, 0, 2]))
        nc.sync.dma_start(wqs[0][:, 1280:2048], wq_d[0][:, 1280:2048])
        nc.sync.dma_start(wqs[2][:, 1280:2048], wq_d[2][:, 1280:2048])
        nc.sync.dma_start(xtv[:, 10:12, :], xT_d[10:12].transpose([1, 0, 2]))
        nc.sync.dma_start(xtv[:, 12:14, :], xT_d[12:14].transpose([1, 0, 2]))
        nc.sync.dma_start(xtv[:, 14:16, :], xT_d[14:16].transpose([1, 0, 2]))
        nc.sync.dma_start(tblf[:], tblf_d[:])
        nc.sync.dma_start(wqs[1][:], wq_d[1])
        nc.sync.dma_start(wqs[3][:], wq_d[3])
        nc.sync.dma_start(cqa[:], cqa_d[:])
        nc.sync.dma_start(sqa[:], sqa_d[:])
        nc.sync.dma_start(cqb[:], cqb_d[:])
        nc.sync.dma_start(sqb[:], sqb_d[:])
        nc.sync.dma_start(wv_sb[:], wv_d[:])
        nc.sync.dma_start(ckt[:], ckt_d[:])
        nc.sync.dma_start(skt[:], skt_d[:])
        nc.sync.dma_start(wk_sb[:], wk_d[:])
        nc.sync.dma_start(tblh[:], tblh_d[:])
        nc.sync.dma_start(msk[:], msk_d[:])
        for g in range(G):
            nc.sync.dma_start(wo_sb[g][:], wo_d[g])

        # ---------------- phase-1 pools ----------------
        # (SBUF pools stay open through attention: the deferred chunk-1
        # DVE filler ops allocate their temporaries from them)
        sbs = pool(name="sb_stream", bufs=2)
        rsp = pool(name="ropes", bufs=2)
        ps1 = ExitStack()
        psproj = ps1.enter_context(tc.tile_pool(name="ps_proj", bufs=6, space="PSUM"))
        ps_v = ps1.enter_context(tc.tile_pool(name="ps_v", bufs=2, space="PSUM"))

        def project4(wa, wb):
            """d-outer accumulation: psums[(fi, c)] = [128, CW] f32."""
            pss = {(fi, c): psproj.tile([128, CW], F32, tag="proj", name="proj")
                   for fi in range(2) for c in range(NCHUNK)}
            for d_i in range(ND):
                for c in range(NCHUNK):
                    for fi, w in enumerate((wa, wb)):
                        nc.tensor.matmul(
                            pss[(fi, c)][:],
                            w[:, d_i * 128:(d_i + 1) * 128],
                            xt_ap(d_i)[:, c * CW:(c + 1) * CW],
                            start=(d_i == 0), stop=(d_i == ND - 1))
            return pss

        def q_copies(pss):
            pcs = {}
            for c in range(NCHUNK):
                pca = sbs.tile([128, CW], F16, tag="pca", name="pca", bufs=4)
                pcb = sbs.tile([128, CW], F16, tag="pcb", name="pcb", bufs=4)
                nc.scalar.copy(pca[:], pss[(0, c)][:])
                nc.scalar.copy(pcb[:], pss[(1, c)][:])
                pcs[c] = (pca, pcb)
            return pcs

        def q_prefix(pca, pcb):
            """sumsq halves -> Pool all-reduce -> Act sqrt.

            Head ga lives in partitions 0:64 of BOTH pca and pcb; gb in
            64:128.  The gpsimd all-reduce only works from base partition
            0, so the gb half is copied down first.
            """
            sqt = sbs.tile([128, CW], F16, tag="sq", name="sq")
            ssum = sbs.tile([128, CW], F16, tag="ssum", name="ssum")
            nc.vector.tensor_mul(sqt[:], pca[:], pca[:])
            nc.vector.tensor_mul(ssum[:], pcb[:], pcb[:])
            nc.vector.tensor_add(ssum[:], ssum[:], sqt[:])
            shi = sbs.tile([64, CW], F16, tag="shi", name="shi")
            nc.vector.tensor_copy(shi[:], ssum[64:128, :])
            ssqa = sbs.tile([64, CW], F16, tag="ssqa", name="ssqa")
            ssqb = sbs.tile([64, CW], F16, tag="ssqb", name="ssqb")
            nc.gpsimd.partition_all_reduce(ssqa[:], ssum[0:64, :],
                                           channels=64, reduce_op=RADD)
            nc.gpsimd.partition_all_reduce(ssqb[:], shi[:],
                                           channels=64, reduce_op=RADD)
            sra = sbs.tile([64, CW], F16, tag="sra", name="sra", bufs=3)
            srb = sbs.tile([64, CW], F16, tag="srb", name="srb", bufs=3)
            nc.scalar.activation(sra[:], ssqa[:],
                                 mybir.ActivationFunctionType.Sqrt,
                                 bias=qbias[0:64, :], scale=float(1.0 / H))
            nc.scalar.activation(srb[:], ssqb[:],
                                 mybir.ActivationFunctionType.Sqrt,
                                 bias=qbias[0:64, :], scale=float(1.0 / H))
            return sra, srb

        def q_rope_ops(pca, pcb, sra, srb, ga, gb, c, pool_muls=False):
            """DVE tail of the q post-chain, as single-op closures:
            rope (independent of the rstd round-trip, so it leads),
            reciprocals, base-64 lift, and the rstd multiplies.
            pool_muls moves the gb-half multiplies to the Pool engine --
            only worth it for the deferred chains popped while the DVE
            is saturated with attention work."""
            gb_eng = nc.gpsimd if pool_muls else nc.vector
            cs = slice(c * CW, (c + 1) * CW)
            st = {}

            def f1():
                st["m1"] = sbs.tile([128, CW], F16, tag="m1", name="m1")
                nc.vector.tensor_mul(st["m1"][:], pca[:], cqa[:, cs])

            def f2():
                st["m2"] = sbs.tile([128, CW], F16, tag="m2", name="m2")
                nc.vector.tensor_mul(st["m2"][:], pcb[:], sqb[:, cs])

            def f3():
                st["ra"] = rsp.tile([128, CW], F16, tag="ra", name="ra")
                nc.vector.tensor_sub(st["ra"][:], st["m1"][:], st["m2"][:])

            def f4():
                nc.vector.tensor_mul(st["m1"][:], pcb[:], cqb[:, cs])

            def f5():
                nc.vector.tensor_mul(st["m2"][:], pca[:], sqa[:, cs])

            def f6():
                st["rb"] = rsp.tile([128, CW], F16, tag="rb", name="rb")
                nc.vector.tensor_add(st["rb"][:], st["m1"][:], st["m2"][:])

            def f7():
                st["rsta"] = sbs.tile([64, CW], F16, tag="rsta", name="rsta")
                nc.vector.reciprocal(st["rsta"][:], sra[:])

            def f8():
                st["rstb"] = sbs.tile([64, CW], F16, tag="rstb", name="rstb")
                nc.vector.reciprocal(st["rstb"][:], srb[:])

            def f9():
                st["rsthi"] = sbs.tile([128, CW], F16, tag="rsthi",
                                       name="rsthi")
                nc.vector.tensor_copy(st["rsthi"][64:128, :], st["rstb"][:])

            qhv = qhvs[c]
            tbs = slice(0, 4)
            r3 = lambda ap: ap.rearrange("p (a t) -> p a t", a=4)

            def f10():
                nc.vector.tensor_mul(qhv[0:64, tbs, ga, :],
                                     r3(st["ra"][0:64, :]),
                                     r3(st["rsta"][:]))

            def f11():
                nc.vector.tensor_mul(qhv[64:128, tbs, ga, :],
                                     r3(st["rb"][0:64, :]),
                                     r3(st["rsta"][:]))

            def f12():
                gb_eng.tensor_mul(qhv[0:64, tbs, gb, :],
                                  r3(st["ra"][64:128, :]),
                                  r3(st["rsthi"][64:128, :]))

            def f13():
                gb_eng.tensor_mul(qhv[64:128, tbs, gb, :],
                                  r3(st["rb"][64:128, :]),
                                  r3(st["rsthi"][64:128, :]))

            return [f1, f2, f3, f4, f5, f6, f7, f8, f9, f10, f11, f12, f13]

        def kv_proj(w, c):
            psk = psproj.tile([128, CW], F32, tag="proj", name="proj")
            for d_i in range(ND):
                nc.tensor.matmul(psk[:], w[:, d_i * 128:(d_i + 1) * 128],
                                 xt_ap(d_i)[:, c * CW:(c + 1) * CW],
                                 start=(d_i == 0), stop=(d_i == ND - 1))
            return psk

        def k_prefix(pck):
            sqk = sbs.tile([128, CW], F16, tag="sqk", name="sqk")
            nc.vector.tensor_mul(sqk[:], pck[:], pck[:])
            kssr = sbs.tile([128, CW], F16, tag="kssr", name="kssr")
            nc.gpsimd.partition_all_reduce(kssr[:], sqk[:],
                                           channels=128, reduce_op=RADD)
            ksq = sbs.tile([128, CW], F16, tag="ksq", name="ksq")
            nc.scalar.activation(ksq[:], kssr[:],
                                 mybir.ActivationFunctionType.Sqrt,
                                 bias=kbias, scale=1.0)
            return ksq

        def k_rope_ops(pck, ksq, c, pool_muls=False):
            cs = slice(c * CW, (c + 1) * CW)
            kT = kTns[c]
            eng = nc.gpsimd if pool_muls else nc.vector
            st = {}
            k0, k1 = pck[0:64, :], pck[64:128, :]

            def f1():
                st["m1"] = sbs.tile([128, CW], F16, tag="km1", name="km1")
                eng.tensor_mul(st["m1"][0:64, :], k0, ckt[0:64, cs])

            def f2():
                st["m2"] = sbs.tile([128, CW], F16, tag="km2", name="km2")
                eng.tensor_mul(st["m2"][0:64, :], k1, skt[64:128, cs])

            def f3():
                nc.vector.tensor_sub(kT[0:64, :], st["m1"][0:64, :],
                                     st["m2"][0:64, :])

            def f4():
                st["m3"] = sbs.tile([128, CW], F16, tag="km3", name="km3")
                eng.tensor_mul(st["m3"][0:64, :], k1, ckt[64:128, cs])

            def f5():
                st["m4"] = sbs.tile([128, CW], F16, tag="km4", name="km4")
                eng.tensor_mul(st["m4"][0:64, :], k0, skt[0:64, cs])

            def f6():
                nc.vector.tensor_add(kT[64:128, :], st["m3"][0:64, :],
                                     st["m4"][0:64, :])

            def f7():
                st["krst"] = sbs.tile([128, CW], F16, tag="krst", name="krst")
                nc.vector.reciprocal(st["krst"][:], ksq[:])

            def f8():
                nc.vector.tensor_mul(kT[:], kT[:], st["krst"][:])

            return [f1, f2, f3, f4, f5, f6, f7, f8]

        # ---------------- q pair A ----------------
        pssA = project4(wqs[0], wqs[2])
        pcsA = q_copies(pssA)
        sraA0, srbA0 = q_prefix(*pcsA[0])
        for f in q_rope_ops(*pcsA[0], sraA0, srbA0, 0, 1, 0):
            f()
        sraA1, srbA1 = q_prefix(*pcsA[1])
        for f in q_rope_ops(*pcsA[1], sraA1, srbA1, 0, 1, 1):
            f()

        # ---------------- q pair B ----------------
        pssB = project4(wqs[1], wqs[3])
        pcsB = q_copies(pssB)
        sraB0, srbB0 = q_prefix(*pcsB[0])
        for f in q_rope_ops(*pcsB[0], sraB0, srbB0, 2, 3, 0):
            f()

        # ---------------- v: chunk-0 projection + PE transposes ----------
        psv0 = kv_proj(wv_sb, 0)
        nc.scalar.copy(vt0[:], psv0[:])
        for jj in range(4):
            vp = ps_v.tile([128, 128], F16, tag="pv", name="pv")
            nc.tensor.transpose(vp[:], vt0[:, jj * 128:(jj + 1) * 128], iden)
            nc.scalar.copy(Vs[0][:, jj * 128:(jj + 1) * 128], vp[:])

        # ---------------- k ----------------
        # both chunk prefixes run up front: ksq1 is the last sqrt-set
        # activation, and the warm exp (table switch) waits on it -- its
        # chain must not queue behind attention-phase Pool/DVE work
        psk0 = kv_proj(wk_sb, 0)
        psk1 = kv_proj(wk_sb, 1)
        pck0 = sbs.tile([128, CW], F16, tag="pck", name="pck")
        nc.scalar.copy(pck0[:], psk0[:])
        pck1 = sbs.tile([128, CW], F16, tag="pck", name="pck")
        nc.scalar.copy(pck1[:], psk1[:])
        ksq0 = k_prefix(pck0)
        ksq1 = k_prefix(pck1)
        for f in k_rope_ops(pck0, ksq0, 0):
            f()
        sraB1, srbB1 = q_prefix(*pcsB[1])   # rope for B-c1 deferred
        # k-c1 rope/fold deferred to the attention-phase DVE fillers

        # switch the Act table to the exp set now: every sqrt-set
        # activation has been emitted above; reading the last k sqrt
        # output places this after all of them
        warm = sbs.tile([1, 2], F16, tag="warm", name="warm")
        nc.scalar.activation(warm[:], ksq1[0:1, 0:2],
                             mybir.ActivationFunctionType.Exp,
                             bias=zcol[0:1, :], scale=-1.0)

        # deferred chunk-1 DVE work, popped between attention ops
        dve_fillers = (q_rope_ops(*pcsB[1], sraB1, srbB1, 2, 3, 1,
                                    pool_muls=True)
                       + k_rope_ops(pck1, ksq1, 1, pool_muls=True))
        dstate = [0]

        def pop_dve(n):
            while n > 0 and dstate[0] < len(dve_fillers):
                dve_fillers[dstate[0]]()
                dstate[0] += 1
                n -= 1

        ps1.close()

        # ---------------- attention + out-projection per t-block ----------------
        sbP = pool(name="sbP", bufs=5)
        sbD = pool(name="sbD", bufs=2)
        sbx = pool(name="sbx", bufs=3)
        osp = pool(name="outs", bufs=3)
        ps_lg = pool(name="ps_lg", bufs=3, space="PSUM")
        ps_qkv = pool(name="ps_qkv", bufs=1, space="PSUM")
        ps_op = pool(name="ps_op", bufs=2, space="PSUM")
        ps_vc1 = pool(name="ps_vc1", bufs=1, space="PSUM")
        ps_pv = pool(name="ps_pv", bufs=1, space="PSUM")

        # deferred v chunk-1 projection: 16 matmuls used as PE filler
        # between the first softmax chains (their latency would otherwise
        # idle the PE before any out-projection work exists)
        psv1 = ps_vc1.tile([128, CW], F32, tag="vc1", name="vc1")
        pe_fillers = []
        for d_i in range(ND):
            def _mk(d):
                def f():
                    nc.tensor.matmul(
                        psv1[:], wv_sb[:, d * 128:(d + 1) * 128],
                        xt_ap(d)[:, CW:2 * CW],
                        start=(d == 0), stop=(d == ND - 1))
                return f
            pe_fillers.append(_mk(d_i))

        def _vt1_copy():
            nc.scalar.copy(vt1[:], psv1[:])
        pe_fillers.append(_vt1_copy)
        fstate = [0]

        def pop_fill(n):
            while n > 0 and fstate[0] < len(pe_fillers):
                pe_fillers[fstate[0]]()
                fstate[0] += 1
                n -= 1

        def v_c1_transposes():
            pop_fill(len(pe_fillers))
            pop_dve(len(dve_fillers))
            for jj in range(4):
                vp = ps_pv.tile([128, 128], F16, tag="pv1", name="pv1")
                nc.tensor.transpose(vp[:], vt1[:, jj * 128:(jj + 1) * 128],
                                    iden)
                nc.scalar.copy(Vs[1][:, jj * 128:(jj + 1) * 128], vp[:])

        # mask-column offset of each (tb, i) partial block, in pack order
        moff = {}
        off = 0
        for tb in range(NTB):
            for i in range(len(plan[tb])):
                if not full[tb][i]:
                    moff[(tb, i)] = off
                    off += 1

        def outproj_dc(tb, qkvh, ob, dc, flush):
            t0 = tb * 128
            op = ps_op.tile([128, CW], F32, tag="op", name="op")
            for g in range(G):
                nc.tensor.matmul(op[:],
                                 qkvh[:, g * 128:(g + 1) * 128],
                                 wo_sb[g][:, dc * CW:(dc + 1) * CW],
                                 start=(g == 0), stop=(g == G - 1))
            if dc % 2 == 0:
                nc.vector.tensor_copy(ob[:, dc * CW:(dc + 1) * CW], op[:])
            else:
                nc.scalar.copy(ob[:, dc * CW:(dc + 1) * CW], op[:])
            if flush:
                nc.sync.dma_start(
                    out_d[t0:t0 + 128, dc * CW:(dc + 1) * CW],
                    ob[:, dc * CW:(dc + 1) * CW])
            elif dc == 3:
                nc.sync.dma_start(out_d[t0:t0 + 128, :], ob[:])

        # pending out-projection units: each is one (tb, dc) 4-matmul
        # group; keeping up to one tb's worth pending lets outproj work
        # fill softmax-chain latency across TWO later t-blocks
        pending = []
        flush_tbs = set()

        def emit_pending(n):
            while n > 0 and pending:
                tb_, qk_, ob_, dc_ = pending.pop(0)
                outproj_dc(tb_, qk_, ob_, dc_, tb_ in flush_tbs)
                n -= 1

        # chunk-0-only t-blocks first (their logits/qkv need no chunk-1
        # k/V), biggest first within each group for outproj filler supply;
        # the smallest chunk-0 tb goes last (shortest tail den chain)
        c0_tbs = sorted((t for t in range(NTB) if max(plan[t]) < NTB // 2),
                        key=lambda t: -len(plan[t]))
        c1_tbs = sorted((t for t in range(NTB) if max(plan[t]) >= NTB // 2),
                        key=lambda t: -len(plan[t]))
        tail_tb = [c0_tbs.pop()] if len(c0_tbs) > 1 else []
        tb_order = c0_tbs + c1_tbs + tail_tb
        flush_tbs.update(tb_order[-2:])

        pop_fill(6)
        did_vc1 = False
        for tb in tb_order:
            ent = plan[tb]
            nv = len(ent)
            if not did_vc1 and max(ent) >= NTB // 2:
                v_c1_transposes()
                did_vc1 = True
            qkvps = ps_qkv.tile([128, 512], F32, tag="qkvp", name="qkvp")
            qkvh = sbx.tile([128, 512], F16, tag="qkvh", name="qkvh")
            ob = osp.tile([128, D], F16, tag="ob", name="ob")
            acc = (sbD.tile([128, 512], F16, tag="dacc", name="dacc")
                   if nv > 1 else None)

            Ps = {}
            for i, si in enumerate(ent):
                # logits for all 4 heads of this (tb, si) block
                lg = ps_lg.tile([128, 512], F32, tag="lg", name="lg")
                sc, sj = divmod(si, NTB // 2)
                tc_, tj = divmod(tb, NTB // 2)
                nc.tensor.matmul(lg[:], kTns[sc][:, sj * 128:(sj + 1) * 128],
                                 qhps[tc_][:, tj * 512:(tj + 1) * 512],
                                 start=True, stop=True)
                P = sbP.tile([128, 512], F16, tag="P", name="P")
                nc.scalar.activation(P[:], lg[:],
                                     mybir.ActivationFunctionType.Exp,
                                     bias=expb, scale=1.0)
                if (tb, i) in moff:
                    mo = moff[(tb, i)]
                    nc.vector.tensor_mul(P[:], P[:],
                                         msk[:, mo * 512:(mo + 1) * 512])
                Ps[i] = P
                if pending:
                    emit_pending(1)
                    pop_fill(1)
                else:
                    pop_fill(3)
                nc.tensor.matmul(qkvps[:], Vs[sc][:, sj * 128:(sj + 1) * 128],
                                 P[:], start=(i == 0), stop=(i == nv - 1))
                if i == 1:
                    nc.vector.tensor_add(acc[:], Ps[0][:], P[:])
                elif i > 1:
                    nc.vector.tensor_add(acc[:], acc[:], P[:])
                pop_dve(2)
            den_src = acc if nv > 1 else Ps[0]
            den = sbD.tile([128, 512], F16, tag="den", name="den")
            nc.gpsimd.partition_all_reduce(den[:], den_src[:],
                                           channels=128, reduce_op=RADD)
            rec = sbD.tile([128, 512], F16, tag="recg", name="recg")
            nc.vector.reciprocal(rec[:], den[:])
            nc.vector.tensor_mul(qkvh[:], qkvps[:], rec[:])
            pop_dve(1)
            Ps.clear()
            for dc in range(4):
                pending.append((tb, qkvh, ob, dc))
            while len(pending) > 8:
                emit_pending(1)
        emit_pending(len(pending))

    nc.finalize()
    return nc


_CACHE = {}


def kernel(x, segment_ids, Wq, Wk, Wv, Wo, q_scale, k_scale):
    global LAST_RESULTS
    import os

    x = np.asarray(x, np.float32)
    seg = np.asarray(segment_ids)
    Wq = np.asarray(Wq, np.float32)
    Wk = np.asarray(Wk, np.float32)
    Wv = np.asarray(Wv, np.float32)
    Wo = np.asarray(Wo, np.float32)
    q_scale = np.asarray(q_scale, np.float32)
    k_scale = np.asarray(k_scale, np.float32)

    plan, full, masks = _classify([seg[b] for b in range(B)])
    key = repr((plan, full))
    if key not in _CACHE:
        _CACHE[key] = _build_nc(plan, full, masks[0].shape[1])
    nc = _CACHE[key]

    half = H // 2
    timescale = ROPE_BASE ** (2.0 * np.arange(half, dtype=np.float64) / H)
    qscA = np.tile(q_scale[:64], 2).astype(np.float64)[:, None]
    qscB = np.tile(q_scale[64:], 2).astype(np.float64)[:, None]
    kvec = k_scale.astype(np.float64)[:, None]
    tabs = []  # per batch: (cqa, sqa, cqb, sqb, ckt, skt)
    for b in range(B):
        pos = _positions(seg[b])
        sinus = pos[:, None] / timescale[None, :]
        sT = np.sin(sinus).T
        cT = np.cos(sinus).T
        c2 = np.vstack([cT, cT])
        s2 = np.vstack([sT, sT])
        tabs.append(tuple(
            np.ascontiguousarray(a, np.float16)
            for a in (c2 * qscA, s2 * qscA, c2 * qscB, s2 * qscB,
                      c2 * kvec, s2 * kvec)))

    tblf = np.zeros((128, 4), np.float32)
    tblf[:, 0] = H * EPS
    tblf[:, 1] = EPS
    tblf[:, 2] = EXPB
    tblf[:, 3] = 0.0
    tblh = np.ascontiguousarray(np.eye(128, dtype=np.float16))

    in_maps = []
    for core in range(8):
        b, kv = core // K, core % K
        qcols = []
        for hv in range(2):
            for g4 in range(G):
                base = kv * 512 + g4 * 128 + hv * 64
                qcols.extend(range(base, base + 64))
        qp = np.array(qcols)
        wq_t = np.ascontiguousarray(
            Wq[:, qp].reshape(ND, 128, G, 128).transpose(2, 1, 0, 3)
            .reshape(G, 128, ND * 128), np.float16)
        wk_t = np.ascontiguousarray(
            Wk[:, kv * 128:(kv + 1) * 128].reshape(ND, 128, 128)
            .transpose(1, 0, 2).reshape(128, ND * 128), np.float16)
        wv_t = np.ascontiguousarray(
            Wv[:, kv * 128:(kv + 1) * 128].reshape(ND, 128, 128)
            .transpose(1, 0, 2).reshape(128, ND * 128), np.float16)
        wo_t = np.ascontiguousarray(
            Wo[kv * 512:(kv + 1) * 512].reshape(G, 128, D), np.float16)
        xt_t = np.ascontiguousarray(
            x[b].T.reshape(ND, 128, T), np.float16)
        cqa, sqa, cqb, sqb, ckt, skt = tabs[b]
        in_maps.append({
            "xT": xt_t, "wq": wq_t, "wk": wk_t, "wv": wv_t, "wo": wo_t,
            "cqa": cqa, "sqa": sqa, "cqb": cqb, "sqb": sqb,
            "ckt": ckt, "skt": skt,
            "tblf": tblf, "tblh": tblh, "masks": masks[b],
        })

    do_trace = os.environ.get("BASS_TRACE") == "1"
    res = run_bass_kernel_spmd(
        nc, in_maps, core_ids=list(range(8)), trace=do_trace)
    LAST_RESULTS = res

    out = np.zeros((B, T, D), np.float32)
    for core in range(8):
        out[core // K] += res.results[core]["out"].astype(np.float32)
    return out
